# revision 24
# baseline (speedup 1.0000x reference)
"""Trainium2 Bass kernel for nn_MoEsparseRoutingForClassification.

Reference computation (B=64, S=128, H=1024, E=8, L=2):
    x = X[:, 0, :]                                   # CLS token [B,H]
    y[b,o]   = sum_e g[b,e] * (x[b] . dense_w[e,o,:]) + (g @ dense_b)[b,o]
    t        = tanh(y)
    out[b,l] = sum_e g[b,e] * (t[b] . out_w[e,l,:])  + (g @ out_b)[b,l]

Distribution: the H output dim of the dense layer is sharded 8 ways
(OC=128 per core).  Core c computes y[:, c*OC:(c+1)*OC] (full CLS token,
but only the slice dense_w[:, c_slice, :]), applies tanh, and contracts
its slice against out_w[:, :, c_slice] into a partial [B,L] logit; the
partials sum on the host (which also adds the tiny gates@out_b bias).
No cross-core collective.

Measured HW model this kernel is built around (v5):
 - The 16 HW DMA engines drain all queues' descriptors in ARRIVAL order
   at ~360 GB/s aggregate (22.5 B/ns/engine cap), and a transfer's
   completion semaphore reaches 16 only ~0.7us after its last byte.
   Skinny descriptors (<2KB/partition) throttle the stream ramp, so the
   CLS token is packed INTO the w1 tensor (one fat 17KB/partition
   stream in 4 chunks the PE chases) and everything else rides one ep
   pack whose rows are ~6KB.  The PE pair cadence is ~2.3x faster once
   the DMA stream stops competing for SBUF ports, so the last two
   k-tiles get their own small chunks (see _CH).
 - dense_w + CLS travel as bf16 (fp32 PSUM accumulation; end-to-end
   scaled error ~2.3e-3 vs the 2e-2 gate); halves the dominant stream.
 - sel_db is folded into the stage-1 PSUM accumulation as two
   contraction-1 matmuls (ones[1,64] x db[1,512]); the gate-broadcast
   table gb[p,(hc,e)] = g[b,e] is produced by the PE (gt2 x
   identity-mask, both packed in ep) instead of 8 DVE broadcast ops.
 - psum_y free-dim layout is (hc, e) so the gate mix is ONE DVE mult
   (bf16 product) + ONE contiguous innermost-axis tensor_reduce.
 - The [128,2] result is PE-transposed to [2,128] (identity built once
   on GpSimd via affine_select) so the out DMA is 2 fat descriptors
   instead of 128x8B - the completion-semaphore chain was ~2us.
 - sel_ob is added on the host (1K MACs).
"""

import sys

import numpy as np
from ml_dtypes import bfloat16 as _BF16

for _p in ("/opt/trn_rl_repo",):
    if _p not in sys.path:
        sys.path.insert(0, _p)

# If the environment sets BASS_TRACE but lacks antenv.axon_hooks (this agent
# image does), run_bass_kernel_spmd would crash on import; pre-seed a no-op
# module so tracing degrades gracefully instead.
try:  # pragma: no cover
    import antenv.axon_hooks  # noqa: F401
except Exception:  # pragma: no cover
    import types as _types

    _m = _types.ModuleType("antenv.axon_hooks")
    _m._hook = None
    _m.set_axon_ntff_profile_hook = lambda h: setattr(_m, "_hook", h)
    _m.get_axon_ntff_profile_hook = lambda: _m._hook
    sys.modules["antenv.axon_hooks"] = _m

B, S, H = 64, 128, 1024
E, L = 8, 2
NCORES = 8
OC = H // NCORES          # dense-output slice per core (128)
HC = OC // 2              # half-slice mapped to a PSUM partition half (64)
KT = H // 128             # contraction tiles
P = 128

XW = KT * B               # xt columns in the packed stream (512 bf16)
WW = KT * 2 * HC * E      # w1 columns (8192 bf16)
# chunk boundaries in packed bf16 columns: xt+k0-2 | k3-5 | k6 | k7.
# The PE runs ~2.3x faster once the DMA stream stops competing for SBUF
# ports, so the last two k-tiles get their own small chunks: k6's
# completion fires before the PE finishes k3-5, and only k7 rides the
# final stream-end semaphore (+0.9us completion lag).
_CH = (0, XW + 3 * 2 * HC * E, XW + 6 * 2 * HC * E, XW + 7 * 2 * HC * E,
       XW + WW)

_cached = None


def _build():
    from contextlib import ExitStack

    import concourse.tile as tile
    from concourse import bacc, mybir

    F32 = mybir.dt.float32
    BF16 = mybir.dt.bfloat16
    AF = mybir.ActivationFunctionType
    OP = mybir.AluOpType

    nc = bacc.Bacc("TRN2", target_bir_lowering=False, debug=False,
                   num_devices=NCORES)

    # E-pack along the free dim (one DMA for all E-partition consts):
    #   gt [E,B] | db2 [E, 512 f32] (row 0 = both h's bf16 bias rows) |
    #   ow2 [E,2,L,HC] | gt2 [E,P] | mask [E,512]
    DB2 = HC * E                         # 512 fp32 cols = 1024 bf16
    MSK = HC * E
    EPACK = B + DB2 + L * OC + P + MSK   # 64+512+256+128+512 = 1472
    wx_d = nc.dram_tensor("wx", [P, XW + WW], BF16, kind="ExternalInput")
    ep_d = nc.dram_tensor("ep", [2 * E, EPACK], F32, kind="ExternalInput")
    out_d = nc.dram_tensor("out", [L, P], F32, kind="ExternalOutput")

    with tile.TileContext(nc) as tc, ExitStack() as ctx:
        consts = ctx.enter_context(tc.tile_pool(name="consts", bufs=1))
        wpool = ctx.enter_context(tc.tile_pool(name="wpool", bufs=1))
        mixp = ctx.enter_context(tc.tile_pool(name="mixp", bufs=2))
        smallp = ctx.enter_context(tc.tile_pool(name="smallp", bufs=1))
        psy = ctx.enter_context(tc.tile_pool(name="psy", bufs=1, space="PSUM"))
        pss = ctx.enter_context(tc.tile_pool(name="pss", bufs=1, space="PSUM"))

        # Trigger order is completion order (per-engine FIFO): tiny ep
        # first, then the fat xt+w1 train the PE chases.
        ep_t16 = consts.tile([2 * E, EPACK], F32)
        nc.sync.dma_start(out=ep_t16, in_=ep_d.ap())
        ep_t = ep_t16[0:E]
        wx_t = wpool.tile([P, XW + WW], BF16)
        for lo, hi in zip(_CH[:-1], _CH[1:]):
            nc.sync.dma_start(out=wx_t[:, lo:hi], in_=wx_d.ap()[:, lo:hi])
        xt_t = wx_t[:, 0:XW].rearrange("p (k b) -> p k b", k=KT)
        w1_t = wx_t[:, XW:].rearrange("p (k h c e) -> p k h c e", k=KT, h=2,
                                      c=HC)
        o = 0
        gt_t = ep_t[:, o:o + B]; o += B
        db2_f32 = ep_t[:, o:o + DB2]; o += DB2
        ow_t = ep_t[:, o:o + L * OC].rearrange(
            "e (h l c) -> e h l c", h=2, l=L); o += L * OC
        gt2_t = ep_t[:, o:o + P]; o += P
        msk_t = ep_t[:, o:o + MSK]; o += MSK

        # ---- early PE work (gated only on ep) ----
        # gb[p, (hc, e)] = g[b, e] via gt2 x mask; sel_ow via gt x ow.
        psum_gb = pss.tile([P, HC, E], F32)
        nc.tensor.matmul(psum_gb[:, :, :].rearrange("p c e -> p (c e)"),
                         gt2_t, msk_t, start=True, stop=True,
                         skip_group_check=True)
        psum_ow = pss.tile([P, L, HC], F32)
        for h in range(2):
            sl = slice(h * 64, h * 64 + 64)
            nc.tensor.matmul(
                psum_ow[sl, :, :].rearrange("b l c -> b (l c)"),
                gt_t, ow_t[:, h].rearrange("e l c -> e (l c)"),
                start=True, stop=True, skip_group_check=True,
            )
        gb_t = consts.tile([P, HC, E], F32)
        nc.scalar.copy(gb_t[:], psum_gb[:])
        sow_t = smallp.tile([P, L, HC], F32)
        nc.scalar.copy(sow_t[:], psum_ow[:])

        # Identity for the final PE transpose, built once on GpSimd.
        id_t = consts.tile([P, P], F32)
        nc.gpsimd.memset(id_t[:], 1.0)
        nc.gpsimd.affine_select(out=id_t[:], in_=id_t[:],
                                pattern=[[-1, P]],
                                compare_op=OP.is_equal, fill=0.0,
                                base=0, channel_multiplier=1)

        # ---- stage 1: y[64h+b, (hc, e)] = x . dense_w + db (bias row) ----
        # Two contraction-1 matmuls seed each h-half's PSUM accumulation
        # with db[e, hc]; the mix then yields y + sel_db exactly.
        ones_bf = smallp.tile([1, B], BF16)
        nc.vector.memset(ones_bf[:], 1.0)
        psum_y = psy.tile([P, HC, E], F32)
        for h in range(2):
            db2_h = db2_f32[0:1, 256 * h:256 * (h + 1)].bitcast(BF16)
            nc.tensor.matmul(
                psum_y[h * 64:h * 64 + 64, :, :].rearrange("b c e -> b (c e)"),
                ones_bf[:], db2_h,
                start=True, stop=False, skip_group_check=True,
            )
        # h=0 / h=1 matmuls write PSUM partition halves -> disjoint PE
        # col-groups run concurrently.  k-outer so the PE consumes each wx
        # chunk as it lands.
        for k in range(KT):
            for h in range(2):
                nc.tensor.matmul(
                    psum_y[h * 64:h * 64 + 64, :, :].rearrange(
                        "b c e -> b (c e)"),
                    xt_t[:, k, :],
                    w1_t[:, k, h].rearrange("p c e -> p (c e)"),
                    start=False,
                    stop=(k == KT - 1),
                    skip_group_check=True,
                )

        # ---- gate mix: one mult + one contiguous innermost reduce ----
        # (GPSIMD cannot access PSUM, so the mult stays whole on the DVE.)
        prod_t = mixp.tile([P, HC, E], F32)
        nc.vector.tensor_tensor(
            out=prod_t[:], in0=psum_y[:], in1=gb_t[:], op=OP.mult,
        )
        # Reduce split across engines: DVE X-reduces hc 0:48 while GpSimd
        # (SBUF-only) runs a pairwise e-tree on hc 48:64; tanh joins both.
        HS = 48
        acc = mixp.tile([P, HC], F32)
        nc.vector.tensor_reduce(acc[:, 0:HS], prod_t[:, 0:HS],
                                axis=mybir.AxisListType.X, op=OP.add)
        tr1 = mixp.tile([P, HC - HS, E // 2], F32)
        nc.gpsimd.tensor_add(tr1[:], prod_t[:, HS:, 0:4], prod_t[:, HS:, 4:8])
        tr2 = mixp.tile([P, HC - HS, E // 4], F32)
        nc.gpsimd.tensor_add(tr2[:], tr1[:, :, 0:2], tr1[:, :, 2:4])
        nc.gpsimd.tensor_add(acc[:, HS:], tr2[:, :, 0], tr2[:, :, 1])

        t_t = smallp.tile([P, HC], F32)
        nc.scalar.activation(t_t[:], acc[:], AF.Tanh)

        # ---- stage 2: partial[64h+b, l] = sum_hc t * sel_ow ----
        # NOTE: InstTensorTensorReduce faults TRN2; scalar_tensor_tensor with
        # accum_out (free-dim sum) is the reliable path (DVE only).
        pre_t = smallp.tile([P, L], F32)
        dump0 = smallp.tile([P, HC], F32)
        for l in range(L):
            nc.vector.scalar_tensor_tensor(
                out=dump0[:], in0=sow_t[:, l, :], scalar=1.0, in1=t_t[:],
                op0=OP.mult, op1=OP.mult, accum_out=pre_t[:, l:l + 1],
            )

        # PE-transpose [128,2] -> [2,128] so the out DMA is 2 descriptors.
        psum_t = pss.tile([P, P], F32)
        nc.tensor.matmul(psum_t[0:L, :], pre_t[:], id_t[:],
                         start=True, stop=True, is_transpose=True,
                         skip_group_check=True)
        ot2 = smallp.tile([L, P], F32)
        nc.scalar.copy(ot2[:], psum_t[0:L, :])

        # Out trigger on the sync ring (the Act engine's DGE descriptor
        # generation is ~2x slower; measured 1188 vs 646 ns).
        nc.sync.dma_start(out=out_d.ap(), in_=ot2[:])

    nc.compile()
    return nc


def _prep_inputs(X, gates, dense_w, dense_b, out_w, out_b):
    """Host-side layout prep (slice/transpose/cast) -> per-core input maps."""
    X = np.asarray(X, dtype=np.float32)
    gates = np.asarray(gates, dtype=np.float32)
    dense_w = np.asarray(dense_w, dtype=np.float32)
    dense_b = np.asarray(dense_b, dtype=np.float32)
    out_w = np.asarray(out_w, dtype=np.float32)

    xcls = X[:, 0, :]                                     # [B, H]
    # xt[i_lo, k, b] = x[b, k*128 + i_lo]
    xt = np.ascontiguousarray(
        xcls.T.reshape(KT, P, B).transpose(1, 0, 2)).astype(_BF16)
    gt = np.ascontiguousarray(gates.T)                    # [E, B]
    gt2 = np.concatenate([gt, gt], axis=1)                # [E, 128]
    # mask[e', hc*8+e] = (e == e')
    mask = np.ascontiguousarray(np.tile(np.eye(E, dtype=np.float32), (1, HC)))

    in_maps = []
    for c in range(NCORES):
        sl = slice(c * OC, (c + 1) * OC)
        # w1[i_lo, k, h, hc, e] = dense_w[e, c*OC + h*64 + hc, k*128 + i_lo]
        w1 = np.ascontiguousarray(
            dense_w[:, sl, :]                   # [E, OC, H]
            .reshape(E, 2, HC, KT, P)           # [e, h, hc, k, i_lo]
            .transpose(4, 3, 1, 2, 0)           # [i_lo, k, h, hc, e]
        ).astype(_BF16)
        wx = np.ascontiguousarray(
            np.concatenate([xt.reshape(P, XW), w1.reshape(P, WW)], axis=1))

        # db2 row 0: [h, hc, e] bias values as bf16, viewed as fp32 words
        db_he = np.ascontiguousarray(
            dense_b[:, sl].reshape(E, 2, HC).transpose(1, 2, 0)  # [h, hc, e]
        ).astype(_BF16).reshape(2 * HC * E)
        db2 = np.zeros((E, HC * E), dtype=np.float32)
        db2[0, :] = db_he.view(np.float32)

        # ow2[e, (h, l, hc)] = out_w[e, l, c*OC + h*64 + hc]
        ow2 = (out_w[:, :, sl].reshape(E, L, 2, HC)
               .transpose(0, 2, 1, 3).reshape(E, L * OC))
        ep = np.ascontiguousarray(
            np.concatenate([gt, db2, ow2, gt2, mask], axis=1),
            dtype=np.float32)
        # 16 rows so every DMA engine warms up on a small first descriptor
        ep = np.concatenate([ep, np.zeros_like(ep)], axis=0)
        in_maps.append({
            "wx": wx,
            "ep": ep,
        })
    return in_maps


def _run(in_maps, trace=False, tmpdir=None):
    global _cached
    from concourse.bass_utils import run_bass_kernel_spmd

    if _cached is None:
        _cached = _build()
    res = run_bass_kernel_spmd(
        _cached, in_maps, list(range(NCORES)), trace=trace, tmpdir=tmpdir,
    )
    return res


def kernel(X, gates, dense_w, dense_b, out_w, out_b):
    in_maps = _prep_inputs(X, gates, dense_w, dense_b, out_w, out_b)
    res = _run(in_maps)
    acc = np.zeros((B, L), dtype=np.float64)
    for c in range(NCORES):
        part = res.results[c]["out"].astype(np.float64)   # [L, 128]
        acc += part.reshape(L, 2, B).sum(axis=1).T        # [B, L]
    # sel_ob = gates @ out_b, added on the host (tiny)
    acc += np.asarray(gates, dtype=np.float64) @ np.asarray(
        out_b, dtype=np.float64)
    return acc.astype(np.float32)


# revision 25
# speedup vs baseline: 1.0228x; 1.0228x over previous
"""Trainium2 Bass kernel for nn_MoEsparseRoutingForClassification.

Reference computation (B=64, S=128, H=1024, E=8, L=2):
    x = X[:, 0, :]                                   # CLS token [B,H]
    y[b,o]   = sum_e g[b,e] * (x[b] . dense_w[e,o,:]) + (g @ dense_b)[b,o]
    t        = tanh(y)
    out[b,l] = sum_e g[b,e] * (t[b] . out_w[e,l,:])  + (g @ out_b)[b,l]

Distribution: the H output dim of the dense layer is sharded 8 ways
(OC=128 per core).  Core c computes y[:, c*OC:(c+1)*OC] (full CLS token,
but only the slice dense_w[:, c_slice, :]), applies tanh, and contracts
its slice against out_w[:, :, c_slice] into a partial [B,L] logit; the
partials sum on the host (which also adds the tiny gates@out_b bias).
No cross-core collective.

Measured HW model this kernel is built around (v5):
 - The 16 HW DMA engines drain all queues' descriptors in ARRIVAL order
   at ~360 GB/s aggregate (22.5 B/ns/engine cap), and a transfer's
   completion semaphore reaches 16 only ~0.7us after its last byte.
   Skinny descriptors (<2KB/partition) throttle the stream ramp, so the
   CLS token is packed INTO the w1 tensor (one fat 17KB/partition
   stream in 4 chunks the PE chases) and everything else rides one ep
   pack whose rows are ~6KB.  The PE pair cadence is ~2.3x faster once
   the DMA stream stops competing for SBUF ports, so the last two
   k-tiles get their own small chunks (see _CH).
 - dense_w + CLS travel as bf16 (fp32 PSUM accumulation; end-to-end
   scaled error ~2.3e-3 vs the 2e-2 gate); halves the dominant stream.
 - sel_db is folded into the stage-1 PSUM accumulation as two
   contraction-1 matmuls (ones[1,64] x db[1,512]); the gate-broadcast
   table gb[p,(hc,e)] = g[b,e] is produced by the PE (gt2 x
   identity-mask, both packed in ep) instead of 8 DVE broadcast ops.
 - psum_y free-dim layout is (hc, e) so the gate mix is ONE DVE mult
   (bf16 product) + ONE contiguous innermost-axis tensor_reduce.
 - The [128,2] result is PE-transposed to [2,128] (identity built once
   on GpSimd via affine_select) so the out DMA is 2 fat descriptors
   instead of 128x8B - the completion-semaphore chain was ~2us.
 - sel_ob is added on the host (1K MACs).
"""

import sys

import numpy as np
from ml_dtypes import bfloat16 as _BF16

for _p in ("/opt/trn_rl_repo",):
    if _p not in sys.path:
        sys.path.insert(0, _p)

# If the environment sets BASS_TRACE but lacks antenv.axon_hooks (this agent
# image does), run_bass_kernel_spmd would crash on import; pre-seed a no-op
# module so tracing degrades gracefully instead.
try:  # pragma: no cover
    import antenv.axon_hooks  # noqa: F401
except Exception:  # pragma: no cover
    import types as _types

    _m = _types.ModuleType("antenv.axon_hooks")
    _m._hook = None
    _m.set_axon_ntff_profile_hook = lambda h: setattr(_m, "_hook", h)
    _m.get_axon_ntff_profile_hook = lambda: _m._hook
    sys.modules["antenv.axon_hooks"] = _m

B, S, H = 64, 128, 1024
E, L = 8, 2
NCORES = 8
OC = H // NCORES          # dense-output slice per core (128)
HC = OC // 2              # half-slice mapped to a PSUM partition half (64)
KT = H // 128             # contraction tiles
P = 128

XW = KT * B               # xt columns in the packed stream (512 bf16)
WW = KT * 2 * HC * E      # w1 columns (8192 bf16)
# chunk boundaries in packed bf16 columns: xt+k0-2 | k3-5 | k6 | k7.
# The PE runs ~2.3x faster once the DMA stream stops competing for SBUF
# ports, so the last two k-tiles get their own small chunks: k6's
# completion fires before the PE finishes k3-5, and only k7 rides the
# final stream-end semaphore (+0.9us completion lag).
_CH = (0, XW + 3 * 2 * HC * E, XW + 6 * 2 * HC * E, XW + 7 * 2 * HC * E,
       XW + WW)

_cached = None


def _build():
    from contextlib import ExitStack

    import concourse.tile as tile
    from concourse import bacc, mybir

    F32 = mybir.dt.float32
    BF16 = mybir.dt.bfloat16
    AF = mybir.ActivationFunctionType
    OP = mybir.AluOpType

    nc = bacc.Bacc("TRN2", target_bir_lowering=False, debug=False,
                   num_devices=NCORES)

    # E-pack along the free dim (one DMA for all E-partition consts):
    #   gt [E,B] | db2 [E, 512 f32] (row 0 = both h's bf16 bias rows) |
    #   ow2 [E,2,L,HC] | gt2 [E,P] | mask [E,512]
    DB2 = HC * E                         # 512 fp32 cols = 1024 bf16
    MSK = HC * E
    EPACK = B + DB2 + L * OC + P + MSK   # 64+512+256+128+512 = 1472
    wx_d = nc.dram_tensor("wx", [P, XW + WW], BF16, kind="ExternalInput")
    ep_d = nc.dram_tensor("ep", [2 * E, EPACK], F32, kind="ExternalInput")
    out_d = nc.dram_tensor("out", [L, P], F32, kind="ExternalOutput")

    with tile.TileContext(nc) as tc, ExitStack() as ctx:
        consts = ctx.enter_context(tc.tile_pool(name="consts", bufs=1))
        wpool = ctx.enter_context(tc.tile_pool(name="wpool", bufs=1))
        mixp = ctx.enter_context(tc.tile_pool(name="mixp", bufs=2))
        smallp = ctx.enter_context(tc.tile_pool(name="smallp", bufs=1))
        psy = ctx.enter_context(tc.tile_pool(name="psy", bufs=1, space="PSUM"))
        pss = ctx.enter_context(tc.tile_pool(name="pss", bufs=1, space="PSUM"))

        # Trigger order is completion order (per-engine FIFO): tiny ep
        # first, then the fat xt+w1 train the PE chases.
        ep_t16 = consts.tile([2 * E, EPACK], F32)
        nc.sync.dma_start(out=ep_t16, in_=ep_d.ap())
        ep_t = ep_t16[0:E]
        wx_t = wpool.tile([P, XW + WW], BF16)
        for lo, hi in zip(_CH[:-1], _CH[1:]):
            nc.sync.dma_start(out=wx_t[:, lo:hi], in_=wx_d.ap()[:, lo:hi])
        xt_t = wx_t[:, 0:XW].rearrange("p (k b) -> p k b", k=KT)
        w1_t = wx_t[:, XW:].rearrange("p (k h c e) -> p k h c e", k=KT, h=2,
                                      c=HC)
        o = 0
        gt_t = ep_t[:, o:o + B]; o += B
        db2_f32 = ep_t[:, o:o + DB2]; o += DB2
        ow_t = ep_t[:, o:o + L * OC].rearrange(
            "e (h l c) -> e h l c", h=2, l=L); o += L * OC
        gt2_t = ep_t[:, o:o + P]; o += P
        msk_t = ep_t[:, o:o + MSK]; o += MSK

        # ---- early PE work (gated only on ep) ----
        # gb[p, (hc, e)] = g[b, e] via gt2 x mask; sel_ow via gt x ow.
        psum_gb = pss.tile([P, HC, E], F32)
        nc.tensor.matmul(psum_gb[:, :, :].rearrange("p c e -> p (c e)"),
                         gt2_t, msk_t, start=True, stop=True,
                         skip_group_check=True)
        psum_ow = pss.tile([P, L, HC], F32)
        for h in range(2):
            sl = slice(h * 64, h * 64 + 64)
            nc.tensor.matmul(
                psum_ow[sl, :, :].rearrange("b l c -> b (l c)"),
                gt_t, ow_t[:, h].rearrange("e l c -> e (l c)"),
                start=True, stop=True, skip_group_check=True,
            )
        gb_t = consts.tile([P, HC, E], F32)
        nc.scalar.copy(gb_t[:], psum_gb[:])
        sow_t = smallp.tile([P, L, HC], F32)
        nc.scalar.copy(sow_t[:], psum_ow[:])

        # Identity for the final PE transpose, built once on GpSimd.
        id_t = consts.tile([P, P], F32)
        nc.gpsimd.memset(id_t[:], 1.0)
        nc.gpsimd.affine_select(out=id_t[:], in_=id_t[:],
                                pattern=[[-1, P]],
                                compare_op=OP.is_equal, fill=0.0,
                                base=0, channel_multiplier=1)

        # ---- stage 1: y[64h+b, (hc, e)] = x . dense_w + db (bias row) ----
        # Two contraction-1 matmuls seed each h-half's PSUM accumulation
        # with db[e, hc]; the mix then yields y + sel_db exactly.
        ones_bf = smallp.tile([1, B], BF16)
        nc.vector.memset(ones_bf[:], 1.0)
        psum_y = psy.tile([P, HC, E], F32)
        for h in range(2):
            db2_h = db2_f32[0:1, 256 * h:256 * (h + 1)].bitcast(BF16)
            nc.tensor.matmul(
                psum_y[h * 64:h * 64 + 64, :, :].rearrange("b c e -> b (c e)"),
                ones_bf[:], db2_h,
                start=True, stop=False, skip_group_check=True,
            )
        # h=0 / h=1 matmuls write PSUM partition halves -> disjoint PE
        # col-groups run concurrently.  k-outer so the PE consumes each wx
        # chunk as it lands.
        for k in range(KT):
            for h in range(2):
                nc.tensor.matmul(
                    psum_y[h * 64:h * 64 + 64, :, :].rearrange(
                        "b c e -> b (c e)"),
                    xt_t[:, k, :],
                    w1_t[:, k, h].rearrange("p c e -> p (c e)"),
                    start=False,
                    stop=(k == KT - 1),
                    skip_group_check=True,
                )

        # ---- gate mix: one mult + one contiguous innermost reduce ----
        # (GPSIMD cannot access PSUM, so the mult stays whole on the DVE.)
        prod_t = mixp.tile([P, HC, E], F32)
        nc.vector.tensor_tensor(
            out=prod_t[:], in0=psum_y[:], in1=gb_t[:], op=OP.mult,
        )
        # (A DVE/GpSimd split of this reduce was tried and is SLOWER: the
        # GpSimd strided-add tree takes ~970ns end-to-end vs 690 here.)
        acc = mixp.tile([P, HC], F32)
        nc.vector.tensor_reduce(acc[:], prod_t[:], axis=mybir.AxisListType.X,
                                op=OP.add)

        t_t = smallp.tile([P, HC], F32)
        nc.scalar.activation(t_t[:], acc[:], AF.Tanh)

        # ---- stage 2: partial[64h+b, l] = sum_hc t * sel_ow ----
        # NOTE: InstTensorTensorReduce faults TRN2; scalar_tensor_tensor with
        # accum_out (free-dim sum) is the reliable path (DVE only).
        pre_t = smallp.tile([P, L], F32)
        dump0 = smallp.tile([P, HC], F32)
        for l in range(L):
            nc.vector.scalar_tensor_tensor(
                out=dump0[:], in0=sow_t[:, l, :], scalar=1.0, in1=t_t[:],
                op0=OP.mult, op1=OP.mult, accum_out=pre_t[:, l:l + 1],
            )

        # PE-transpose [128,2] -> [2,128] so the out DMA is 2 descriptors.
        psum_t = pss.tile([P, P], F32)
        nc.tensor.matmul(psum_t[0:L, :], pre_t[:], id_t[:],
                         start=True, stop=True, is_transpose=True,
                         skip_group_check=True)
        ot2 = smallp.tile([L, P], F32)
        nc.scalar.copy(ot2[:], psum_t[0:L, :])

        # Out trigger on the sync ring (the Act engine's DGE descriptor
        # generation is ~2x slower; measured 1188 vs 646 ns).
        nc.sync.dma_start(out=out_d.ap(), in_=ot2[:])

    nc.compile()
    return nc


def _prep_inputs(X, gates, dense_w, dense_b, out_w, out_b):
    """Host-side layout prep (slice/transpose/cast) -> per-core input maps."""
    X = np.asarray(X, dtype=np.float32)
    gates = np.asarray(gates, dtype=np.float32)
    dense_w = np.asarray(dense_w, dtype=np.float32)
    dense_b = np.asarray(dense_b, dtype=np.float32)
    out_w = np.asarray(out_w, dtype=np.float32)

    xcls = X[:, 0, :]                                     # [B, H]
    # xt[i_lo, k, b] = x[b, k*128 + i_lo]
    xt = np.ascontiguousarray(
        xcls.T.reshape(KT, P, B).transpose(1, 0, 2)).astype(_BF16)
    gt = np.ascontiguousarray(gates.T)                    # [E, B]
    gt2 = np.concatenate([gt, gt], axis=1)                # [E, 128]
    # mask[e', hc*8+e] = (e == e')
    mask = np.ascontiguousarray(np.tile(np.eye(E, dtype=np.float32), (1, HC)))

    in_maps = []
    for c in range(NCORES):
        sl = slice(c * OC, (c + 1) * OC)
        # w1[i_lo, k, h, hc, e] = dense_w[e, c*OC + h*64 + hc, k*128 + i_lo]
        w1 = np.ascontiguousarray(
            dense_w[:, sl, :]                   # [E, OC, H]
            .reshape(E, 2, HC, KT, P)           # [e, h, hc, k, i_lo]
            .transpose(4, 3, 1, 2, 0)           # [i_lo, k, h, hc, e]
        ).astype(_BF16)
        wx = np.ascontiguousarray(
            np.concatenate([xt.reshape(P, XW), w1.reshape(P, WW)], axis=1))

        # db2 row 0: [h, hc, e] bias values as bf16, viewed as fp32 words
        db_he = np.ascontiguousarray(
            dense_b[:, sl].reshape(E, 2, HC).transpose(1, 2, 0)  # [h, hc, e]
        ).astype(_BF16).reshape(2 * HC * E)
        db2 = np.zeros((E, HC * E), dtype=np.float32)
        db2[0, :] = db_he.view(np.float32)

        # ow2[e, (h, l, hc)] = out_w[e, l, c*OC + h*64 + hc]
        ow2 = (out_w[:, :, sl].reshape(E, L, 2, HC)
               .transpose(0, 2, 1, 3).reshape(E, L * OC))
        ep = np.ascontiguousarray(
            np.concatenate([gt, db2, ow2, gt2, mask], axis=1),
            dtype=np.float32)
        # 16 rows so every DMA engine warms up on a small first descriptor
        ep = np.concatenate([ep, np.zeros_like(ep)], axis=0)
        in_maps.append({
            "wx": wx,
            "ep": ep,
        })
    return in_maps


def _run(in_maps, trace=False, tmpdir=None):
    global _cached
    from concourse.bass_utils import run_bass_kernel_spmd

    if _cached is None:
        _cached = _build()
    res = run_bass_kernel_spmd(
        _cached, in_maps, list(range(NCORES)), trace=trace, tmpdir=tmpdir,
    )
    return res


def kernel(X, gates, dense_w, dense_b, out_w, out_b):
    in_maps = _prep_inputs(X, gates, dense_w, dense_b, out_w, out_b)
    res = _run(in_maps)
    acc = np.zeros((B, L), dtype=np.float64)
    for c in range(NCORES):
        part = res.results[c]["out"].astype(np.float64)   # [L, 128]
        acc += part.reshape(L, 2, B).sum(axis=1).T        # [B, L]
    # sel_ob = gates @ out_b, added on the host (tiny)
    acc += np.asarray(gates, dtype=np.float64) @ np.asarray(
        out_b, dtype=np.float64)
    return acc.astype(np.float32)
